# revision 84
# baseline (speedup 1.0000x reference)
"""Trainium2 Bass kernel for nn_CDEM_62079457296798 (channel-attention
transformer block).

Sharding: 8 cores = 4 batches x 2 spatial halves (64 rows + 1 halo row each).
Cross-core communication: one small AllReduce per core-pair carrying the
channel-attention Gram matrices and q/k l2-norm sums; everything else local.

Layout: channel-major activations [C_part, pixels_free]. The attention path
(q/kv convs, depthwise 3x3, Gram, z) uses per-head channel padding 48 -> 64
(256 padded channels) so head boundaries are 32/64 aligned, and runs in bf16.
The depthwise 3x3 runs on the tensor engine as 9 accumulated 128-wide
diagonal matmuls per chunk, tap-outer over chunk pairs. Each input image is
streamed once per block-pair; conv passes are emission-interleaved with the
previous dw block (and the p=0 Gram) so the tensor queue never drains. The
ffn1/ffn2 GEMMs run in fp8e4m3 with DoubleRow pair contraction (96+96 K
bands); v/attnT are fp8; the trunk is a 2-stage software pipeline. qT/kT
transposes and sq-sums run on the Act HWDGE ring / DVE+GpSimd off the
critical dp-drain path. Output is written bf16 and upcast on the host.
"""
import sys
sys.path.insert(0, '/opt/trn_rl_repo')

import numpy as np
import ml_dtypes

from concourse import bacc, mybir, tile
from concourse.ap import AP
from concourse.bass_utils import run_bass_kernel_spmd

F32 = mybir.dt.float32
F32R = mybir.dt.float32r
BF16 = mybir.dt.bfloat16
F8 = mybir.dt.float8e4
AF = mybir.ActivationFunctionType
OP = mybir.AluOpType
bf16 = ml_dtypes.bfloat16
f8np = mybir.dt.np(F8)
DR = mybir.MatmulPerfMode.DoubleRow

N_CORES = 8
B, C, H, W = 4, 192, 128, 128
HEADS, CH = 4, 48
CPH = 64                # padded channels per head
CP = HEADS * CPH        # 256 padded attn channels
HLOC = 64               # image rows per core
ER, EC = 66, 130        # ext rows/cols (halo + zero pad)
NEXT = ER * EC          # 8580
NLOC = HLOC * W         # 8192
NCK = 16                # trunk output chunks (4 rows x 128 = 512 px)
NCKD = 8                # dw output chunks (8 rows x 128 = 1024 px)
CONV_CHUNKS = [(i * 1024, 1024) for i in range(8)] + [(8192, NEXT - 8192)]
KB = [(0, 128), (128, 64)]          # 192-channel K bands

DIRECT_PSUM_OUT = False  # DMA final result straight from PSUM


import os
STAGE = int(os.environ.get("KSTAGE", "4"))
KSUB = int(os.environ.get("KSUB", "4"))


class _StageDone(Exception):
    pass


def build_nc():
    nc = bacc.Bacc("TRN2", target_bir_lowering=False, debug=False,
                   num_devices=N_CORES)

    d_xe = nc.dram_tensor("xe", [C, NEXT], BF16, kind="ExternalInput")
    d_ye = nc.dram_tensor("ye", [C, NEXT], BF16, kind="ExternalInput")
    d_yc = nc.dram_tensor("yc", [C, NLOC], BF16, kind="ExternalInput")
    d_wq = nc.dram_tensor("wq", [C, CP], BF16, kind="ExternalInput")
    d_wkv = nc.dram_tensor("wkv", [C, 2 * CP], BF16, kind="ExternalInput")
    d_qdw = nc.dram_tensor("qdw", [CP, 9, 128], BF16, kind="ExternalInput")
    d_kvdw = nc.dram_tensor("kvdw", [2 * CP, 9, 128], BF16, kind="ExternalInput")
    d_wlin = nc.dram_tensor("wlin", [CP, C], BF16, kind="ExternalInput")
    d_wf1 = nc.dram_tensor("wf1", [96, 2, 768], F8, kind="ExternalInput")
    d_wf2 = nc.dram_tensor("wf2", [3, 128, 2, C], F8, kind="ExternalInput")
    d_wpr = nc.dram_tensor("wpr", [C, C], BF16, kind="ExternalInput")
    d_tempb = nc.dram_tensor("tempb", [128, 2], F32, kind="ExternalInput")
    d_alpha = nc.dram_tensor("alpha", [128, 1], F32, kind="ExternalInput")
    d_gamma = nc.dram_tensor("gamma", [128, 1], F32, kind="ExternalInput")
    d_id128 = nc.dram_tensor("id128", [128, 128], F32, kind="ExternalInput")
    d_idrep = nc.dram_tensor("idrep", [128, 64], BF16, kind="ExternalInput")
    d_out = nc.dram_tensor("out", [C, NLOC], BF16, kind="ExternalOutput")
    cc_in = nc.dram_tensor("cc_in", [112, 228], F32)
    cc_out = nc.dram_tensor("cc_out", [112, 228], F32)

    with tile.TileContext(nc) as tc:
        with (
            tc.tile_pool(name="sbw", bufs=1) as sbw,      # weights/consts
            tc.tile_pool(name="sbpre", bufs=3) as sbpre,  # conv1x1 out (ext img)
            tc.tile_pool(name="sbin", bufs=4) as sbin,    # streamed conv inputs
            tc.tile_pool(name="sbqk", bufs=7) as sbqk,    # q/k chunk tiles
            tc.tile_pool(name="sbdq", bufs=1) as sbdq,    # Square dump tile
            tc.tile_pool(name="sbT", bufs=1) as sbT,      # qT/kT/v persistents
            tc.tile_pool(name="sbs", bufs=1) as sbs,      # small attn tiles
            tc.tile_pool(name="sbc", bufs=2) as sbc,      # trunk chunk pipeline
            tc.tile_pool(name="sbg", bufs=2) as sbg,      # fp8 gelu pair tiles
            tc.tile_pool(name="pcv", bufs=4, space="PSUM") as pcv,
            tc.tile_pool(name="pdw", bufs=2, space="PSUM") as pdw,
            tc.tile_pool(name="pacc", bufs=1, space="PSUM") as pacc,
            tc.tile_pool(name="psm", bufs=1, space="PSUM") as psm,
        ):
            # ---------- weights ----------
            wq_t = [sbw.tile([s, CP], BF16, tag=f"wq{i}", name=f"wq{i}")
                    for i, (o, s) in enumerate(KB)]
            wkv_t = [sbw.tile([s, 2 * CP], BF16, tag=f"wkv{i}", name=f"wkv{i}")
                     for i, (o, s) in enumerate(KB)]
            for i, (o, s) in enumerate(KB):
                nc.sync.dma_start(wq_t[i][:], d_wq[o:o + s, :])
            qdw_t = [sbw.tile([128, 9, 128], BF16, tag=f"qdw{m}", name=f"qdw{m}") for m in range(2)]
            kvdw_t = [sbw.tile([128, 9, 128], BF16, tag=f"kvdw{m}", name=f"kvdw{m}") for m in range(4)]
            for m in range(2):
                nc.sync.dma_start(qdw_t[m][:], d_qdw[128 * m:128 * (m + 1)])

            def load_kv_weights():
                for i, (o, s) in enumerate(KB):
                    nc.sync.dma_start(wkv_t[i][:], d_wkv[o:o + s, :])
                for m in range(4):
                    nc.sync.dma_start(kvdw_t[m][:], d_kvdw[128 * m:128 * (m + 1)])
            wlin_t = [sbw.tile([128, C], BF16, tag=f"wlin{m}", name=f"wlin{m}") for m in range(2)]
            wf1p = sbw.tile([96, 2, 768], F8, tag="wf1p", name="wf1p")
            wf2p = [sbw.tile([128, 2, C], F8, tag=f"wf2p{g}", name=f"wf2p{g}")
                    for g in range(3)]
            wpr_t = [sbw.tile([96, C], BF16, tag=f"wpr{i}", name=f"wpr{i}")
                     for i in range(2)]
            tempb = sbw.tile([128, 2], F32, tag="tempb", name="tempb")
            alphav = sbw.tile([128, 1], F32, tag="alphav", name="alphav")
            gammav = sbw.tile([128, 1], F32, tag="gammav", name="gammav")
            id128 = sbw.tile([128, 128], F32, tag="id128", name="id128")
            idrep = sbw.tile([128, 64], BF16, tag="idrep", name="idrep")

            def load_trunk_weights():
                for m in range(2):
                    nc.sync.dma_start(wlin_t[m][:], d_wlin[128 * m:128 * (m + 1), :])
                nc.sync.dma_start(wf1p[:], d_wf1.ap())
                for g in range(3):
                    nc.sync.dma_start(wf2p[g][:], d_wf2[g])
                for i in range(2):
                    nc.sync.dma_start(wpr_t[i][:], d_wpr[96 * i:96 * (i + 1), :])
                nc.sync.dma_start(tempb[:], d_tempb.ap())
                nc.sync.dma_start(alphav[:], d_alpha.ap())
                nc.sync.dma_start(gammav[:], d_gamma.ap())
                nc.sync.dma_start(id128[:], d_id128.ap())
                nc.sync.dma_start(idrep[:], d_idrep.ap())

            # persistent attn-path results
            qT = [sbT.tile([128, 64, 112], BF16, tag=f"qT{p}", name=f"qT{p}") for p in range(2)]
            kT = [sbT.tile([128, 64, 112], BF16, tag=f"kT{p}", name=f"kT{p}") for p in range(2)]
            vband = [sbT.tile([128, NLOC], F8, tag=f"v{m}", name=f"v{m}") for m in range(2)]
            sqp = [sbs.tile([128, NCK], F32, tag=f"sqp{i}", name=f"sqp{i}") for i in range(4)]
            for i in range(4):
                nc.vector.memset(sqp[i][:], 0.0)
            gacc = pacc.tile([112, 224], F32, tag="gacc", name="gacc")

            # ============ q/k/v production ============
            def mk_pres(nm):
                return [sbpre.tile([128, ER, EC], BF16, tag="pre",
                                   name=f"pre{nm}{j}") for j in range(2)]

            def conv_pair_gen(src_dram, w_t, m0, pres):
                """conv1x1 for blocks m0, m0+1 streaming the input once."""
                prefs = [p[:].rearrange("p a b -> p (a b)") for p in pres]
                nco = 0
                for ci, (c0, cn) in enumerate(CONV_CHUNKS):
                    xc = [sbin.tile([s, 1024], BF16, tag=f"xin{i}", name=f"xin{i}")
                          for i, (o, s) in enumerate(KB)]
                    for i, (o, s) in enumerate(KB):
                        nc.sync.dma_start(xc[i][:, :cn],
                                          src_dram[o:o + s, c0:c0 + cn])
                    for j in range(2):
                        m = m0 + j
                        for h0 in range(0, cn, 512):
                            hn = min(512, cn - h0)
                            ps = pcv.tile([128, 512], F32, tag="cv", name="cv")
                            for i in range(2):
                                nc.tensor.matmul(ps[:, :hn],
                                                 w_t[i][:, 128 * m:128 * (m + 1)],
                                                 xc[i][:, h0:h0 + hn],
                                                 start=(i == 0), stop=(i == 1))
                            nc.vector.tensor_copy(
                                prefs[j][:, c0 + h0:c0 + h0 + hn], ps[:, :hn])
                            nco += 1
                    yield

            _SENT = object()

            def interleave(gen_a, gen_b, nb=1):
                """Round-robin emission: one chunk of a, nb chunks of b."""
                while True:
                    a_live = next(gen_a, _SENT) is not _SENT
                    b_live = False
                    for _ in range(nb):
                        if next(gen_b, _SENT) is not _SENT:
                            b_live = True
                    if not a_live and not b_live:
                        break

            def drain(gen):
                for _ in gen:
                    pass

            def dw_gen(m, pre, dw_tiles, sink):
                # tap-outer over chunk pairs: back-to-back matmuls share the
                # same stationary tap weights
                for ck2 in range(NCK // 2 if KSUB >= 2 else 0):
                    dps = [pdw.tile([128, 4, 128], F32, tag="dw", name="dw")
                           for _ in range(2)]
                    for t in range(9):
                        dr, dc = t // 3 - 1, t % 3 - 1
                        for u in range(2):
                            r0 = 1 + 4 * (2 * ck2 + u)
                            nc.tensor.matmul(
                                dps[u][:, :, :],
                                dw_tiles[m][:, t, :],
                                pre[:, r0 + dr:r0 + 4 + dr, 1 + dc:129 + dc],
                                start=(t == 0), stop=(t == 8))
                    for u in range(2):
                        sink(m, 2 * ck2 + u,
                             dps[u][:].rearrange("p a b -> p (a b)"))
                    yield

            def qk_sink(dstT, sq_idx):
                def sink(m, ck, flat):
                    qc = sbqk.tile([128, 512], BF16, tag="qkc", name="qkc")
                    nc.vector.tensor_copy(qc[:], flat)
                    if KSUB >= 3:
                        dmp = sbdq.tile([128, 512], F32, tag="dump", name="dump")
                        nc.gpsimd.tensor_tensor(out=dmp[:], in0=qc[:], in1=qc[:],
                                                op=OP.mult)
                        nc.vector.tensor_reduce(sqp[sq_idx + m][:, ck:ck + 1],
                                                dmp[:], axis=mybir.AxisListType.X,
                                                op=OP.add)
                    if KSUB >= 4:
                        nc.scalar.dma_start_transpose(
                            dstT[m][:, 4 * ck:4 * ck + 4, :], qc[0:112, :])
                return sink

            pre_q = mk_pres("q")
            gq = conv_pair_gen(d_xe, wq_t, 0, pre_q)
            for _ in range(3):
                next(gq)
            load_kv_weights()
            drain(gq)
            drain(dw_gen(0, pre_q[0], qdw_t, qk_sink(qT, 0)))

            if STAGE >= 2:

                def v_sink(m, ck, flat):
                    dst = vband[m - 2]
                    nc.vector.tensor_copy(dst[:, ck * 512:(ck + 1) * 512], flat)

                pre_k = mk_pres("k")
                interleave(conv_pair_gen(d_ye, wkv_t, 0, pre_k),
                           dw_gen(1, pre_q[1], qdw_t, qk_sink(qT, 0)))
                load_trunk_weights()
                drain(dw_gen(0, pre_k[0], kvdw_t, qk_sink(kT, 2)))
                def gram_gen(p):
                    for ck0 in range(0, 64, 8):
                        for ck in range(ck0, ck0 + 8):
                            nc.tensor.matmul(gacc[:, 112 * p:112 * (p + 1)],
                                             qT[p][:, ck, :], kT[p][:, ck, :],
                                             start=(ck == 0), stop=(ck == 63))
                        yield

                # conv-v + dw-k1 + gram-p0 share the tensor queue round-robin
                # (gram p0 only needs kT[0], complete after dw k0)
                pre_v = mk_pres("v")
                gens = [conv_pair_gen(d_ye, wkv_t, 2, pre_v),
                        dw_gen(1, pre_k[1], kvdw_t, qk_sink(kT, 2)),
                        gram_gen(0)]
                while gens:
                    gens = [g for g in gens if next(g, _SENT) is not _SENT]
                drain(gram_gen(1))
            else:
                drain(dw_gen(1, pre_q[1], qdw_t, qk_sink(qT, 0)))

            if STAGE < 3:
                oc0 = sbs.tile([128, NCK], F32, tag="oc0d", name="oc0d")
                nc.vector.tensor_copy(oc0[:], sqp[0][:])
                nc.sync.dma_start(d_out[0:128, 0:NCK], oc0[:])
            if STAGE >= 3:
                sqv = sbs.tile([128, 2], F32, tag="sqv", name="sqv")
                skv = sbs.tile([128, 2], F32, tag="skv", name="skv")
                for m in range(2):
                    nc.vector.tensor_reduce(sqv[:, m:m + 1], sqp[m][:],
                                            axis=mybir.AxisListType.X, op=OP.add)
                    nc.vector.tensor_reduce(skv[:, m:m + 1], sqp[2 + m][:],
                                            axis=mybir.AxisListType.X, op=OP.add)

                # ============ pair AllReduce ============
                gsb = sbs.tile([112, 224], F32, tag="gsb", name="gsb")
                nc.vector.tensor_copy(gsb[:], gacc[:])
                nc.sync.dma_start(cc_in.ap()[:, 0:224], gsb[:])
                nc.sync.dma_start(cc_in.ap()[:, 224:226], sqv[0:112, :])
                nc.sync.dma_start(cc_in.ap()[:, 226:228], skv[0:112, :])
                nc.gpsimd.collective_compute(
                    "AllReduce", OP.add,
                    replica_groups=[[0, 1], [2, 3], [4, 5], [6, 7]],
                    ins=[cc_in.ap()], outs=[cc_out.ap()])
                # v dw overlaps the AllReduce; the second dw block is emitted
                # after the softmax section so the tensor queue has work while
                # vector/scalar run the softmax chain
                drain(dw_gen(2, pre_v[0], kvdw_t, v_sink))
                gg = sbs.tile([112, 224], F32, tag="gg", name="gg")
                sqg = sbs.tile([128, 2], F32, tag="sqg", name="sqg")
                skg = sbs.tile([128, 2], F32, tag="skg", name="skg")
                nc.vector.memset(sqg[:], 1.0)
                nc.vector.memset(skg[:], 1.0)
                nc.sync.dma_start(gg[:], cc_out.ap()[:, 0:224])
                nc.sync.dma_start(sqg[0:112, :], cc_out.ap()[:, 224:226])
                nc.sync.dma_start(skg[0:112, :], cc_out.ap()[:, 226:228])

                # ============ attention finalize ============
                def rsqrt_newton(tag, s_t):
                    sc = sbs.tile([128, 2], F32, tag=tag + "_c")
                    nc.vector.tensor_scalar_max(sc[:], s_t[:], 1e-24)
                    rt = sbs.tile([128, 2], F32, tag=tag + "_s")
                    nc.scalar.activation(rt[:], sc[:], AF.Sqrt)
                    r0 = sbs.tile([128, 2], F32, tag=tag + "_r0")
                    nc.vector.reciprocal(r0[:], rt[:])
                    rr = sbs.tile([128, 2], F32, tag=tag + "_rr")
                    nc.vector.tensor_tensor(out=rr[:], in0=r0[:], in1=r0[:], op=OP.mult)
                    t1_ = sbs.tile([128, 2], F32, tag=tag + "_t1")
                    nc.vector.scalar_tensor_tensor(out=t1_[:], in0=sc[:], scalar=-0.5,
                                                   in1=rr[:], op0=OP.mult, op1=OP.mult)
                    nc.vector.tensor_scalar_add(t1_[:], t1_[:], 1.5)
                    rv = sbs.tile([128, 2], F32, tag=tag)
                    nc.vector.tensor_tensor(out=rv[:], in0=r0[:], in1=t1_[:], op=OP.mult)
                    return rv

                rq = rsqrt_newton("rq", sqg)
                rk = rsqrt_newton("rk", skg)
                srow = sbs.tile([128, 2], F32, tag="srow", name="srow")
                nc.vector.tensor_tensor(out=srow[:], in0=rq[:], in1=tempb[:], op=OP.mult)

                srow_r, scol_r = [], []
                for p in range(2):
                    for src, lst, nm in ((srow, srow_r, "sr"), (rk, scol_r, "sc")):
                        fp = psm.tile([1, 112], F32, tag="sm", name="sm")
                        nc.tensor.transpose(fp[:], src[0:112, p:p + 1],
                                            id128[0:112, 0:112])
                        fr = sbs.tile([1, 112], F32R, tag=f"{nm}{p}", name=f"{nm}{p}")
                        nc.vector.tensor_copy(fr[:], fp[:])
                        lst.append(fr)

                attnT = [sbs.tile([128, 64], F8, tag=f"attnT{p}", name=f"attnT{p}") for p in range(2)]
                for p in range(2):
                    nc.gpsimd.memset(attnT[p][:], 0.0)
                for p in range(2):
                    spair = psm.tile([112, 112], F32, tag="sm", name="sm")
                    nc.tensor.matmul(spair[:], srow_r[p][:], scol_r[p][:],
                                     start=True, stop=True)
                    lg = sbs.tile([112, 112], F32, tag="lg", name="lg")
                    nc.vector.tensor_tensor(out=lg[:], in0=gg[:, 112 * p:112 * (p + 1)],
                                            in1=spair[:], op=OP.mult)
                    at16 = sbs.tile([112, 112], BF16, tag="at16", name="at16")
                    for e in range(2):
                        sl = slice(64 * e, 64 * e + 48)
                        mx = sbs.tile([112, 1], F32, tag="mx", name="mx")
                        nc.vector.tensor_reduce(mx[sl, :], lg[sl, sl],
                                                axis=mybir.AxisListType.X, op=OP.max)
                        exh = sbs.tile([112, 112], F32, tag="exh", name="exh")
                        nc.vector.tensor_scalar(out=exh[sl, 0:48], in0=lg[sl, sl],
                                                scalar1=mx[sl, :], scalar2=None,
                                                op0=OP.subtract)
                        ex2 = sbs.tile([112, 112], F32, tag="ex2", name="ex2")
                        den = sbs.tile([112, 1], F32, tag="den", name="den")
                        nc.scalar.activation(ex2[sl, 0:48], exh[sl, 0:48], AF.Exp,
                                             accum_out=den[sl, :])
                        rc0 = sbs.tile([112, 1], F32, tag="rc0", name="rc0")
                        nc.vector.reciprocal(rc0[sl, :], den[sl, :])
                        nt = sbs.tile([112, 1], F32, tag="nt", name="nt")
                        nc.vector.tensor_tensor(out=nt[sl, :], in0=den[sl, :],
                                                in1=rc0[sl, :], op=OP.mult)
                        nc.vector.tensor_scalar(out=nt[sl, :], in0=nt[sl, :],
                                                scalar1=-1.0, scalar2=2.0,
                                                op0=OP.mult, op1=OP.add)
                        rc1 = sbs.tile([112, 1], F32, tag="rc1", name="rc1")
                        nc.vector.tensor_tensor(out=rc1[sl, :], in0=rc0[sl, :],
                                                in1=nt[sl, :], op=OP.mult)
                        nc.vector.tensor_scalar(out=at16[sl, 0:48], in0=ex2[sl, 0:48],
                                                scalar1=rc1[sl, :], scalar2=None,
                                                op0=OP.mult)
                        tps = pcv.tile([128, 64], BF16, tag="cv", name="tps")
                        nc.tensor.transpose(tps[sl, 0:48], at16[sl, 0:48],
                                            idrep[sl, 0:48])
                        nc.vector.tensor_copy(attnT[p][sl, 0:48], tps[sl, 0:48])

                if STAGE < 4:
                    ocx = sbs.tile([112, 64], F32, tag="ocx", name="ocx")
                    nc.vector.tensor_copy(ocx[:], attnT[0][0:112, :])
                    nc.sync.dma_start(d_out[0:112, 0:64], ocx[:])
                # second v dw block — tensor work emitted after the softmax chain
                drain(dw_gen(3, pre_v[1], kvdw_t, v_sink))
                # ============ trunk: 2-stage software pipeline ============
                def trunk_A(ck):
                    c0 = ck * 512
                    zp = [pdw.tile([128, 512], F32, tag="dw", name="dw") for _ in range(2)]
                    for p in range(2):
                        for e in range(2):
                            osl = slice(64 * e, 64 * e + 64)
                            nc.tensor.matmul(zp[p][osl, :], attnT[p][osl, :],
                                             vband[p][osl, c0:c0 + 512],
                                             start=True, stop=True,
                                             tile_position=(64 * e, 64 * e))
                    zc = [sbc.tile([128, 512], BF16, tag=f"zc{m}", name=f"zc{m}") for m in range(2)]
                    nc.vector.tensor_copy(zc[0][:], zp[0][:])
                    nc.vector.tensor_copy(zc[1][:], zp[1][:])
                    tp = [pcv.tile([128, 512], F32, tag="cv", name="cv") for _ in range(2)]
                    for mi in range(2):
                        mo = 96 * mi
                        for k2 in range(2):
                            nc.tensor.matmul(tp[mi][:96, :],
                                             wlin_t[k2][:, mo:mo + 96], zc[k2][:],
                                             start=(k2 == 0), stop=(k2 == 1))
                    ycn = sbc.tile([96, 2, 512], BF16, tag="ycn", name="ycn")
                    for mi in range(2):
                        nc.sync.dma_start(ycn[:, mi, :],
                                          d_yc[96 * mi:96 * (mi + 1), c0:c0 + 512])
                    t1c = [sbc.tile([96, 512], BF16, tag=f"t1c{i}", name=f"t1c{i}")
                           for i in range(2)]
                    t1p = sbc.tile([96, 2, 512], F8, tag="t1p", name="t1p")
                    for mi in range(2):
                        nc.vector.scalar_tensor_tensor(
                            out=t1c[mi][:], in0=ycn[:, mi, :], scalar=alphav[:96, :],
                            in1=tp[mi][:96, :], op0=OP.mult, op1=OP.add)
                        nc.vector.tensor_copy(t1p[:, mi, :], t1c[mi][:])
                    return t1c, t1p

                def trunk_B(ck, t1c, t1p):
                    c0 = ck * 512
                    gcp = [sbg.tile([128, 2, 512], F8, tag=f"gcp{g}", name=f"gcp{g}")
                           for g in range(3)]
                    for mt in range(6):
                        fp1 = pcv.tile([128, 512], F32, tag="cv", name="cv")
                        nc.tensor.matmul(fp1[:], wf1p[:, :, 128 * mt:128 * (mt + 1)],
                                         t1p[:], start=True, stop=True,
                                         perf_mode=DR)
                        nc.scalar.activation(gcp[mt % 3][:, mt // 3, :], fp1[:],
                                             AF.Gelu)
                    t2c = [sbc.tile([96, 512], BF16, tag=f"t2c{i}", name=f"t2c{i}")
                           for i in range(2)]
                    for mi in range(2):
                        mo = 96 * mi
                        fp2 = pcv.tile([128, 512], F32, tag="cv", name="cv")
                        for g in range(3):
                            nc.tensor.matmul(fp2[:96, :], wf2p[g][:, :, mo:mo + 96],
                                             gcp[g][:], start=(g == 0), stop=(g == 2),
                                             perf_mode=DR)
                        nc.vector.scalar_tensor_tensor(
                            out=t2c[mi][:], in0=t1c[mi][:], scalar=gammav[:96, :],
                            in1=fp2[:96, :], op0=OP.mult, op1=OP.add)
                    for mi, (mo, ms) in enumerate(KB):
                        pp = pcv.tile([128, 512], F32, tag="cv", name="cv")
                        for i in range(2):
                            nc.tensor.matmul(pp[:ms, :], wpr_t[i][:, mo:mo + ms],
                                             t2c[i][:], start=(i == 0), stop=(i == 1))
                        oc = sbc.tile([128, 512], BF16, tag=f"oc{mi}", name=f"oc{mi}")
                        nc.vector.tensor_copy(oc[:ms, :], pp[:ms, :])
                        nc.sync.dma_start(d_out[mo:mo + ms, c0:c0 + 512],
                                          oc[:ms, :])

                prevA = None
                for ck in range(NCK + 1 if STAGE >= 4 else 0):
                    if ck < NCK:
                        curA = (ck, trunk_A(ck))
                    if prevA is not None:
                        pk, (t1c_, t1p_) = prevA
                        trunk_B(pk, t1c_, t1p_)
                    prevA = curA if ck < NCK else None

    nc.compile()
    return nc


_NC = None


def _get_nc():
    global _NC
    if _NC is None:
        _NC = build_nc()
    return _NC


def _prep_weights(q_w, q_dw_w, kv_w, kv_dw_w, linear_w, proj_w, ffn1_w, ffn2_w,
                  temperature, alpha, beta, gamma, delta):
    def pad_oc(w):  # [192 real oc, ic] -> [ic, 256 padded oc]
        out = np.zeros((C, CP), np.float32)
        for h in range(HEADS):
            out[:, CPH * h:CPH * h + CH] = w[CH * h:CH * (h + 1), :].T
        return out

    wq = pad_oc(np.asarray(q_w, np.float32))
    kv = np.asarray(kv_w, np.float32)
    wkv = np.concatenate([pad_oc(kv[:C]), pad_oc(kv[C:])], axis=1)

    def pad_dw(w):  # [192,1,3,3] -> [256, 9, 128] diag
        out = np.zeros((CP, 9, 128), np.float32)
        for h in range(HEADS):
            for j in range(CH):
                cp = CPH * h + j
                out[cp, :, cp % 128] = w[CH * h + j, 0].reshape(9)
        return out

    qdw = pad_dw(np.asarray(q_dw_w, np.float32))
    kvd = np.asarray(kv_dw_w, np.float32)
    kvdw = np.concatenate([pad_dw(kvd[:C]), pad_dw(kvd[C:])], axis=0)

    lin = np.asarray(linear_w, np.float32) * float(beta)
    wlin = np.zeros((CP, C), np.float32)
    for h in range(HEADS):
        wlin[CPH * h:CPH * h + CH, :] = lin[:, CH * h:CH * (h + 1)].T

    wf1 = np.asarray(ffn1_w, np.float32).T  # [192, 768]
    wf1p = wf1.reshape(2, 96, 768).transpose(1, 0, 2).copy()
    wf2 = (np.asarray(ffn2_w, np.float32) * float(delta)).T  # [768, 192]
    wf2p = wf2.reshape(2, 3, 128, C).transpose(1, 2, 0, 3).copy()
    wpr = np.asarray(proj_w, np.float32).T.copy()

    tempb = np.zeros((128, 2), np.float32)
    tv = np.asarray(temperature, np.float32).reshape(HEADS)
    for h in range(HEADS):
        tempb[64 * (h % 2):64 * (h % 2) + 64, h // 2] = tv[h]

    alphav = np.full((128, 1), float(alpha), np.float32)
    gammav = np.full((128, 1), float(gamma), np.float32)
    id128 = np.eye(128, dtype=np.float32)
    idrep = np.zeros((128, 64), np.float32)
    for p_ in range(128):
        idrep[p_, p_ % 64] = 1.0

    return {
        "wq": wq.astype(bf16), "wkv": wkv.astype(bf16),
        "qdw": qdw.astype(bf16), "kvdw": kvdw.astype(bf16),
        "wlin": wlin.astype(bf16), "wf1": wf1p.astype(f8np), "wf2": wf2p.astype(f8np), "wpr": wpr.astype(bf16),
        "tempb": tempb, "alpha": alphav, "gamma": gammav,
        "id128": id128, "idrep": idrep.astype(bf16),
    }


def kernel(**inputs):
    x = np.asarray(inputs["x"], np.float32)
    y = np.asarray(inputs["y"], np.float32)
    shared = _prep_weights(
        inputs["q_w"], inputs["q_dw_w"], inputs["kv_w"], inputs["kv_dw_w"],
        inputs["linear_w"], inputs["proj_w"], inputs["ffn1_w"], inputs["ffn2_w"],
        inputs["temperature"], inputs["alpha"], inputs["beta"],
        inputs["gamma"], inputs["delta"])

    in_maps = []
    for c in range(N_CORES):
        bi, s = c // 2, c % 2
        r0 = s * HLOC
        xe = np.zeros((C, ER, EC), np.float32)
        ye = np.zeros((C, ER, EC), np.float32)
        rlo, rhi = max(r0 - 1, 0), min(r0 + HLOC + 1, H)
        elo = rlo - (r0 - 1)
        xe[:, elo:elo + (rhi - rlo), 1:129] = x[bi, :, rlo:rhi, :]
        ye[:, elo:elo + (rhi - rlo), 1:129] = y[bi, :, rlo:rhi, :]
        m = dict(shared)
        m["xe"] = xe.reshape(C, NEXT).astype(bf16)
        m["ye"] = ye.reshape(C, NEXT).astype(bf16)
        m["yc"] = y[bi, :, r0:r0 + HLOC, :].reshape(C, NLOC).astype(bf16)
        in_maps.append(m)

    nc = _get_nc()
    res = run_bass_kernel_spmd(nc, in_maps, list(range(N_CORES)))
    out = np.empty((B, C, H, W), np.float32)
    for c in range(N_CORES):
        bi, s = c // 2, c % 2
        out[bi, :, s * HLOC:(s + 1) * HLOC, :] = \
            res.results[c]["out"].reshape(C, HLOC, W)
    return out



# revision 85
# speedup vs baseline: 1.0457x; 1.0457x over previous
"""Trainium2 Bass kernel for nn_CDEM_62079457296798 (channel-attention
transformer block).

Sharding: 8 cores = 4 batches x 2 spatial halves (64 rows + 1 halo row each).
Cross-core communication: one small AllReduce per core-pair carrying the
channel-attention Gram matrices and q/k l2-norm sums; everything else local.

Layout: channel-major activations [C_part, pixels_free]. The attention path
(q/kv convs, depthwise 3x3, Gram, z) uses per-head channel padding 48 -> 64
(256 padded channels) so head boundaries are 32/64 aligned, and runs in bf16.
The depthwise 3x3 runs on the tensor engine as 9 accumulated 128-wide
diagonal matmuls per chunk, tap-outer over chunk pairs. Each input image is
streamed once per block-pair; conv passes are emission-interleaved with the
previous dw block (and the p=0 Gram) so the tensor queue never drains. The
ffn1/ffn2 GEMMs run in fp8e4m3 with DoubleRow pair contraction (96+96 K
bands); v/attnT are fp8; the trunk is a 2-stage software pipeline. qT/kT
transposes and sq-sums run on the Act HWDGE ring / DVE+GpSimd off the
critical dp-drain path. Output is written bf16 and upcast on the host.
"""
import sys
sys.path.insert(0, '/opt/trn_rl_repo')

import numpy as np
import ml_dtypes

from concourse import bacc, mybir, tile
from concourse.ap import AP
from concourse.bass_utils import run_bass_kernel_spmd

F32 = mybir.dt.float32
F32R = mybir.dt.float32r
BF16 = mybir.dt.bfloat16
F8 = mybir.dt.float8e4
AF = mybir.ActivationFunctionType
OP = mybir.AluOpType
bf16 = ml_dtypes.bfloat16
f8np = mybir.dt.np(F8)
DR = mybir.MatmulPerfMode.DoubleRow

N_CORES = 8
B, C, H, W = 4, 192, 128, 128
HEADS, CH = 4, 48
CPH = 64                # padded channels per head
CP = HEADS * CPH        # 256 padded attn channels
HLOC = 64               # image rows per core
ER, EC = 66, 130        # ext rows/cols (halo + zero pad)
NEXT = ER * EC          # 8580
NLOC = HLOC * W         # 8192
NCK = 16                # trunk output chunks (4 rows x 128 = 512 px)
NCKD = 8                # dw output chunks (8 rows x 128 = 1024 px)
CONV_CHUNKS = [(i * 1024, 1024) for i in range(8)] + [(8192, NEXT - 8192)]
KB = [(0, 128), (128, 64)]          # 192-channel K bands

DIRECT_PSUM_OUT = False  # DMA final result straight from PSUM


import os
STAGE = int(os.environ.get("KSTAGE", "4"))
KSUB = int(os.environ.get("KSUB", "4"))


class _StageDone(Exception):
    pass


def build_nc():
    nc = bacc.Bacc("TRN2", target_bir_lowering=False, debug=False,
                   num_devices=N_CORES)

    d_xe = nc.dram_tensor("xe", [C, NEXT], BF16, kind="ExternalInput")
    d_ye = nc.dram_tensor("ye", [C, NEXT], BF16, kind="ExternalInput")
    d_yc = nc.dram_tensor("yc", [C, NLOC], BF16, kind="ExternalInput")
    d_wq = nc.dram_tensor("wq", [C, CP], BF16, kind="ExternalInput")
    d_wkv = nc.dram_tensor("wkv", [C, 2 * CP], BF16, kind="ExternalInput")
    d_qdw = nc.dram_tensor("qdw", [CP, 9, 128], BF16, kind="ExternalInput")
    d_kvdw = nc.dram_tensor("kvdw", [2 * CP, 9, 128], BF16, kind="ExternalInput")
    d_wlin = nc.dram_tensor("wlin", [CP, C], BF16, kind="ExternalInput")
    d_wf1 = nc.dram_tensor("wf1", [96, 2, 768], F8, kind="ExternalInput")
    d_wf2 = nc.dram_tensor("wf2", [3, 128, 2, C], F8, kind="ExternalInput")
    d_wpr = nc.dram_tensor("wpr", [C, C], BF16, kind="ExternalInput")
    d_tempb = nc.dram_tensor("tempb", [128, 2], F32, kind="ExternalInput")
    d_alpha = nc.dram_tensor("alpha", [128, 1], F32, kind="ExternalInput")
    d_gamma = nc.dram_tensor("gamma", [128, 1], F32, kind="ExternalInput")
    d_id128 = nc.dram_tensor("id128", [128, 128], F32, kind="ExternalInput")
    d_idrep = nc.dram_tensor("idrep", [128, 64], BF16, kind="ExternalInput")
    d_out = nc.dram_tensor("out", [C, NLOC], BF16, kind="ExternalOutput")
    cc_in = nc.dram_tensor("cc_in", [112, 228], F32)
    cc_out = nc.dram_tensor("cc_out", [112, 228], F32)

    with tile.TileContext(nc) as tc:
        with (
            tc.tile_pool(name="sbw", bufs=1) as sbw,      # weights/consts
            tc.tile_pool(name="sbpre", bufs=3) as sbpre,  # conv1x1 out (ext img)
            tc.tile_pool(name="sbin", bufs=4) as sbin,    # streamed conv inputs
            tc.tile_pool(name="sbqk", bufs=7) as sbqk,    # q/k chunk tiles
            tc.tile_pool(name="sbdq", bufs=1) as sbdq,    # Square dump tile
            tc.tile_pool(name="sbT", bufs=1) as sbT,      # qT/kT/v persistents
            tc.tile_pool(name="sbs", bufs=1) as sbs,      # small attn tiles
            tc.tile_pool(name="sbc", bufs=2) as sbc,      # trunk chunk pipeline
            tc.tile_pool(name="sbg", bufs=2) as sbg,      # fp8 gelu pair tiles
            tc.tile_pool(name="pcv", bufs=3, space="PSUM") as pcv,
            tc.tile_pool(name="pdw", bufs=3, space="PSUM") as pdw,
            tc.tile_pool(name="pacc", bufs=1, space="PSUM") as pacc,
            tc.tile_pool(name="psm", bufs=1, space="PSUM") as psm,
        ):
            # ---------- weights ----------
            wq_t = [sbw.tile([s, CP], BF16, tag=f"wq{i}", name=f"wq{i}")
                    for i, (o, s) in enumerate(KB)]
            wkv_t = [sbw.tile([s, 2 * CP], BF16, tag=f"wkv{i}", name=f"wkv{i}")
                     for i, (o, s) in enumerate(KB)]
            for i, (o, s) in enumerate(KB):
                nc.sync.dma_start(wq_t[i][:], d_wq[o:o + s, :])
            qdw_t = [sbw.tile([128, 9, 128], BF16, tag=f"qdw{m}", name=f"qdw{m}") for m in range(2)]
            kvdw_t = [sbw.tile([128, 9, 128], BF16, tag=f"kvdw{m}", name=f"kvdw{m}") for m in range(4)]
            for m in range(2):
                nc.sync.dma_start(qdw_t[m][:], d_qdw[128 * m:128 * (m + 1)])

            def load_kv_weights():
                for i, (o, s) in enumerate(KB):
                    nc.sync.dma_start(wkv_t[i][:], d_wkv[o:o + s, :])
                for m in range(4):
                    nc.sync.dma_start(kvdw_t[m][:], d_kvdw[128 * m:128 * (m + 1)])
            wlin_t = [sbw.tile([128, C], BF16, tag=f"wlin{m}", name=f"wlin{m}") for m in range(2)]
            wf1p = sbw.tile([96, 2, 768], F8, tag="wf1p", name="wf1p")
            wf2p = [sbw.tile([128, 2, C], F8, tag=f"wf2p{g}", name=f"wf2p{g}")
                    for g in range(3)]
            wpr_t = [sbw.tile([96, C], BF16, tag=f"wpr{i}", name=f"wpr{i}")
                     for i in range(2)]
            tempb = sbw.tile([128, 2], F32, tag="tempb", name="tempb")
            alphav = sbw.tile([128, 1], F32, tag="alphav", name="alphav")
            gammav = sbw.tile([128, 1], F32, tag="gammav", name="gammav")
            id128 = sbw.tile([128, 128], F32, tag="id128", name="id128")
            idrep = sbw.tile([128, 64], BF16, tag="idrep", name="idrep")

            def load_trunk_weights():
                for m in range(2):
                    nc.sync.dma_start(wlin_t[m][:], d_wlin[128 * m:128 * (m + 1), :])
                nc.sync.dma_start(wf1p[:], d_wf1.ap())
                for g in range(3):
                    nc.sync.dma_start(wf2p[g][:], d_wf2[g])
                for i in range(2):
                    nc.sync.dma_start(wpr_t[i][:], d_wpr[96 * i:96 * (i + 1), :])
                nc.sync.dma_start(tempb[:], d_tempb.ap())
                nc.sync.dma_start(alphav[:], d_alpha.ap())
                nc.sync.dma_start(gammav[:], d_gamma.ap())
                nc.sync.dma_start(id128[:], d_id128.ap())
                nc.sync.dma_start(idrep[:], d_idrep.ap())

            # persistent attn-path results
            qT = [sbT.tile([128, 64, 112], BF16, tag=f"qT{p}", name=f"qT{p}") for p in range(2)]
            kT = [sbT.tile([128, 64, 112], BF16, tag=f"kT{p}", name=f"kT{p}") for p in range(2)]
            vband = [sbT.tile([128, NLOC], F8, tag=f"v{m}", name=f"v{m}") for m in range(2)]
            sqp = [sbs.tile([128, NCK], F32, tag=f"sqp{i}", name=f"sqp{i}") for i in range(4)]
            for i in range(4):
                nc.vector.memset(sqp[i][:], 0.0)
            gacc = pacc.tile([112, 224], F32, tag="gacc", name="gacc")

            # ============ q/k/v production ============
            def mk_pres(nm):
                return [sbpre.tile([128, ER, EC], BF16, tag="pre",
                                   name=f"pre{nm}{j}") for j in range(2)]

            def conv_pair_gen(src_dram, w_t, m0, pres):
                """conv1x1 for blocks m0, m0+1 streaming the input once."""
                prefs = [p[:].rearrange("p a b -> p (a b)") for p in pres]
                nco = 0
                for ci, (c0, cn) in enumerate(CONV_CHUNKS):
                    xc = [sbin.tile([s, 1024], BF16, tag=f"xin{i}", name=f"xin{i}")
                          for i, (o, s) in enumerate(KB)]
                    for i, (o, s) in enumerate(KB):
                        nc.sync.dma_start(xc[i][:, :cn],
                                          src_dram[o:o + s, c0:c0 + cn])
                    for j in range(2):
                        m = m0 + j
                        for h0 in range(0, cn, 512):
                            hn = min(512, cn - h0)
                            ps = pcv.tile([128, 512], F32, tag="cv", name="cv")
                            for i in range(2):
                                nc.tensor.matmul(ps[:, :hn],
                                                 w_t[i][:, 128 * m:128 * (m + 1)],
                                                 xc[i][:, h0:h0 + hn],
                                                 start=(i == 0), stop=(i == 1))
                            nc.vector.tensor_copy(
                                prefs[j][:, c0 + h0:c0 + h0 + hn], ps[:, :hn])
                            nco += 1
                    yield

            _SENT = object()

            def interleave(gen_a, gen_b, nb=1):
                """Round-robin emission: one chunk of a, nb chunks of b."""
                while True:
                    a_live = next(gen_a, _SENT) is not _SENT
                    b_live = False
                    for _ in range(nb):
                        if next(gen_b, _SENT) is not _SENT:
                            b_live = True
                    if not a_live and not b_live:
                        break

            def drain(gen):
                for _ in gen:
                    pass

            def dw_gen(m, pre, dw_tiles, sink):
                # tap-outer over chunk pairs: back-to-back matmuls share the
                # same stationary tap weights
                for ck2 in range(NCK // 2 if KSUB >= 2 else 0):
                    dps = [pdw.tile([128, 4, 128], F32, tag="dw", name="dw")
                           for _ in range(2)]
                    for t in range(9):
                        dr, dc = t // 3 - 1, t % 3 - 1
                        for u in range(2):
                            r0 = 1 + 4 * (2 * ck2 + u)
                            nc.tensor.matmul(
                                dps[u][:, :, :],
                                dw_tiles[m][:, t, :],
                                pre[:, r0 + dr:r0 + 4 + dr, 1 + dc:129 + dc],
                                start=(t == 0), stop=(t == 8))
                    for u in range(2):
                        sink(m, 2 * ck2 + u,
                             dps[u][:].rearrange("p a b -> p (a b)"))
                    yield

            def qk_sink(dstT, sq_idx):
                def sink(m, ck, flat):
                    qc = sbqk.tile([128, 512], BF16, tag="qkc", name="qkc")
                    nc.vector.tensor_copy(qc[:], flat)
                    if KSUB >= 3:
                        dmp = sbdq.tile([128, 512], F32, tag="dump", name="dump")
                        nc.gpsimd.tensor_tensor(out=dmp[:], in0=qc[:], in1=qc[:],
                                                op=OP.mult)
                        nc.vector.tensor_reduce(sqp[sq_idx + m][:, ck:ck + 1],
                                                dmp[:], axis=mybir.AxisListType.X,
                                                op=OP.add)
                    if KSUB >= 4:
                        nc.scalar.dma_start_transpose(
                            dstT[m][:, 4 * ck:4 * ck + 4, :], qc[0:112, :])
                return sink

            pre_q = mk_pres("q")
            gq = conv_pair_gen(d_xe, wq_t, 0, pre_q)
            for _ in range(3):
                next(gq)
            load_kv_weights()
            drain(gq)
            drain(dw_gen(0, pre_q[0], qdw_t, qk_sink(qT, 0)))

            if STAGE >= 2:

                def v_sink(m, ck, flat):
                    dst = vband[m - 2]
                    nc.vector.tensor_copy(dst[:, ck * 512:(ck + 1) * 512], flat)

                pre_k = mk_pres("k")
                interleave(conv_pair_gen(d_ye, wkv_t, 0, pre_k),
                           dw_gen(1, pre_q[1], qdw_t, qk_sink(qT, 0)))
                load_trunk_weights()
                drain(dw_gen(0, pre_k[0], kvdw_t, qk_sink(kT, 2)))
                def gram_gen(p):
                    for ck0 in range(0, 64, 8):
                        for ck in range(ck0, ck0 + 8):
                            nc.tensor.matmul(gacc[:, 112 * p:112 * (p + 1)],
                                             qT[p][:, ck, :], kT[p][:, ck, :],
                                             start=(ck == 0), stop=(ck == 63))
                        yield

                # conv-v + dw-k1 + gram-p0 share the tensor queue round-robin
                # (gram p0 only needs kT[0], complete after dw k0)
                pre_v = mk_pres("v")
                gens = [conv_pair_gen(d_ye, wkv_t, 2, pre_v),
                        dw_gen(1, pre_k[1], kvdw_t, qk_sink(kT, 2)),
                        gram_gen(0)]
                while gens:
                    gens = [g for g in gens if next(g, _SENT) is not _SENT]
                drain(gram_gen(1))
            else:
                drain(dw_gen(1, pre_q[1], qdw_t, qk_sink(qT, 0)))

            if STAGE < 3:
                oc0 = sbs.tile([128, NCK], F32, tag="oc0d", name="oc0d")
                nc.vector.tensor_copy(oc0[:], sqp[0][:])
                nc.sync.dma_start(d_out[0:128, 0:NCK], oc0[:])
            if STAGE >= 3:
                sqv = sbs.tile([128, 2], F32, tag="sqv", name="sqv")
                skv = sbs.tile([128, 2], F32, tag="skv", name="skv")
                for m in range(2):
                    nc.vector.tensor_reduce(sqv[:, m:m + 1], sqp[m][:],
                                            axis=mybir.AxisListType.X, op=OP.add)
                    nc.vector.tensor_reduce(skv[:, m:m + 1], sqp[2 + m][:],
                                            axis=mybir.AxisListType.X, op=OP.add)

                # ============ pair AllReduce ============
                gsb = sbs.tile([112, 224], F32, tag="gsb", name="gsb")
                nc.vector.tensor_copy(gsb[:], gacc[:])
                nc.sync.dma_start(cc_in.ap()[:, 0:224], gsb[:])
                nc.sync.dma_start(cc_in.ap()[:, 224:226], sqv[0:112, :])
                nc.sync.dma_start(cc_in.ap()[:, 226:228], skv[0:112, :])
                nc.gpsimd.collective_compute(
                    "AllReduce", OP.add,
                    replica_groups=[[0, 1], [2, 3], [4, 5], [6, 7]],
                    ins=[cc_in.ap()], outs=[cc_out.ap()])
                # v dw overlaps the AllReduce; the second dw block is emitted
                # after the softmax section so the tensor queue has work while
                # vector/scalar run the softmax chain
                drain(dw_gen(2, pre_v[0], kvdw_t, v_sink))
                gg = sbs.tile([112, 224], F32, tag="gg", name="gg")
                sqg = sbs.tile([128, 2], F32, tag="sqg", name="sqg")
                skg = sbs.tile([128, 2], F32, tag="skg", name="skg")
                nc.vector.memset(sqg[:], 1.0)
                nc.vector.memset(skg[:], 1.0)
                nc.sync.dma_start(gg[:], cc_out.ap()[:, 0:224])
                nc.sync.dma_start(sqg[0:112, :], cc_out.ap()[:, 224:226])
                nc.sync.dma_start(skg[0:112, :], cc_out.ap()[:, 226:228])

                # ============ attention finalize ============
                def rsqrt_newton(tag, s_t):
                    sc = sbs.tile([128, 2], F32, tag=tag + "_c")
                    nc.vector.tensor_scalar_max(sc[:], s_t[:], 1e-24)
                    rt = sbs.tile([128, 2], F32, tag=tag + "_s")
                    nc.scalar.activation(rt[:], sc[:], AF.Sqrt)
                    r0 = sbs.tile([128, 2], F32, tag=tag + "_r0")
                    nc.vector.reciprocal(r0[:], rt[:])
                    rr = sbs.tile([128, 2], F32, tag=tag + "_rr")
                    nc.vector.tensor_tensor(out=rr[:], in0=r0[:], in1=r0[:], op=OP.mult)
                    t1_ = sbs.tile([128, 2], F32, tag=tag + "_t1")
                    nc.vector.scalar_tensor_tensor(out=t1_[:], in0=sc[:], scalar=-0.5,
                                                   in1=rr[:], op0=OP.mult, op1=OP.mult)
                    nc.vector.tensor_scalar_add(t1_[:], t1_[:], 1.5)
                    rv = sbs.tile([128, 2], F32, tag=tag)
                    nc.vector.tensor_tensor(out=rv[:], in0=r0[:], in1=t1_[:], op=OP.mult)
                    return rv

                rq = rsqrt_newton("rq", sqg)
                rk = rsqrt_newton("rk", skg)
                srow = sbs.tile([128, 2], F32, tag="srow", name="srow")
                nc.vector.tensor_tensor(out=srow[:], in0=rq[:], in1=tempb[:], op=OP.mult)

                srow_r, scol_r = [], []
                for p in range(2):
                    for src, lst, nm in ((srow, srow_r, "sr"), (rk, scol_r, "sc")):
                        fp = psm.tile([1, 112], F32, tag="sm", name="sm")
                        nc.tensor.transpose(fp[:], src[0:112, p:p + 1],
                                            id128[0:112, 0:112])
                        fr = sbs.tile([1, 112], F32R, tag=f"{nm}{p}", name=f"{nm}{p}")
                        nc.vector.tensor_copy(fr[:], fp[:])
                        lst.append(fr)

                attnT = [sbs.tile([128, 64], F8, tag=f"attnT{p}", name=f"attnT{p}") for p in range(2)]
                for p in range(2):
                    nc.gpsimd.memset(attnT[p][:], 0.0)
                for p in range(2):
                    spair = psm.tile([112, 112], F32, tag="sm", name="sm")
                    nc.tensor.matmul(spair[:], srow_r[p][:], scol_r[p][:],
                                     start=True, stop=True)
                    lg = sbs.tile([112, 112], F32, tag="lg", name="lg")
                    nc.vector.tensor_tensor(out=lg[:], in0=gg[:, 112 * p:112 * (p + 1)],
                                            in1=spair[:], op=OP.mult)
                    at16 = sbs.tile([112, 112], BF16, tag="at16", name="at16")
                    for e in range(2):
                        sl = slice(64 * e, 64 * e + 48)
                        mx = sbs.tile([112, 1], F32, tag="mx", name="mx")
                        nc.vector.tensor_reduce(mx[sl, :], lg[sl, sl],
                                                axis=mybir.AxisListType.X, op=OP.max)
                        exh = sbs.tile([112, 112], F32, tag="exh", name="exh")
                        nc.vector.tensor_scalar(out=exh[sl, 0:48], in0=lg[sl, sl],
                                                scalar1=mx[sl, :], scalar2=None,
                                                op0=OP.subtract)
                        ex2 = sbs.tile([112, 112], F32, tag="ex2", name="ex2")
                        den = sbs.tile([112, 1], F32, tag="den", name="den")
                        nc.scalar.activation(ex2[sl, 0:48], exh[sl, 0:48], AF.Exp,
                                             accum_out=den[sl, :])
                        rc0 = sbs.tile([112, 1], F32, tag="rc0", name="rc0")
                        nc.vector.reciprocal(rc0[sl, :], den[sl, :])
                        nt = sbs.tile([112, 1], F32, tag="nt", name="nt")
                        nc.vector.tensor_tensor(out=nt[sl, :], in0=den[sl, :],
                                                in1=rc0[sl, :], op=OP.mult)
                        nc.vector.tensor_scalar(out=nt[sl, :], in0=nt[sl, :],
                                                scalar1=-1.0, scalar2=2.0,
                                                op0=OP.mult, op1=OP.add)
                        rc1 = sbs.tile([112, 1], F32, tag="rc1", name="rc1")
                        nc.vector.tensor_tensor(out=rc1[sl, :], in0=rc0[sl, :],
                                                in1=nt[sl, :], op=OP.mult)
                        nc.vector.tensor_scalar(out=at16[sl, 0:48], in0=ex2[sl, 0:48],
                                                scalar1=rc1[sl, :], scalar2=None,
                                                op0=OP.mult)
                        tps = pcv.tile([128, 64], BF16, tag="cv", name="tps")
                        nc.tensor.transpose(tps[sl, 0:48], at16[sl, 0:48],
                                            idrep[sl, 0:48])
                        nc.vector.tensor_copy(attnT[p][sl, 0:48], tps[sl, 0:48])

                if STAGE < 4:
                    ocx = sbs.tile([112, 64], F32, tag="ocx", name="ocx")
                    nc.vector.tensor_copy(ocx[:], attnT[0][0:112, :])
                    nc.sync.dma_start(d_out[0:112, 0:64], ocx[:])
                # second v dw block — tensor work emitted after the softmax chain
                drain(dw_gen(3, pre_v[1], kvdw_t, v_sink))
                # ============ trunk: 2-stage software pipeline ============
                def trunk_A(ck):
                    c0 = ck * 512
                    zp = [pdw.tile([128, 512], F32, tag="dw", name="dw") for _ in range(2)]
                    for p in range(2):
                        for e in range(2):
                            osl = slice(64 * e, 64 * e + 64)
                            nc.tensor.matmul(zp[p][osl, :], attnT[p][osl, :],
                                             vband[p][osl, c0:c0 + 512],
                                             start=True, stop=True,
                                             tile_position=(64 * e, 64 * e))
                    zc = [sbc.tile([128, 512], BF16, tag=f"zc{m}", name=f"zc{m}") for m in range(2)]
                    nc.vector.tensor_copy(zc[0][:], zp[0][:])
                    nc.vector.tensor_copy(zc[1][:], zp[1][:])
                    tp = [pcv.tile([128, 512], F32, tag="cv", name="cv") for _ in range(2)]
                    for mi in range(2):
                        mo = 96 * mi
                        for k2 in range(2):
                            nc.tensor.matmul(tp[mi][:96, :],
                                             wlin_t[k2][:, mo:mo + 96], zc[k2][:],
                                             start=(k2 == 0), stop=(k2 == 1))
                    ycn = sbc.tile([96, 2, 512], BF16, tag="ycn", name="ycn")
                    for mi in range(2):
                        nc.sync.dma_start(ycn[:, mi, :],
                                          d_yc[96 * mi:96 * (mi + 1), c0:c0 + 512])
                    t1c = [sbc.tile([96, 512], BF16, tag=f"t1c{i}", name=f"t1c{i}")
                           for i in range(2)]
                    t1p = sbc.tile([96, 2, 512], F8, tag="t1p", name="t1p")
                    for mi in range(2):
                        nc.vector.scalar_tensor_tensor(
                            out=t1c[mi][:], in0=ycn[:, mi, :], scalar=alphav[:96, :],
                            in1=tp[mi][:96, :], op0=OP.mult, op1=OP.add)
                        nc.vector.tensor_copy(t1p[:, mi, :], t1c[mi][:])
                    return t1c, t1p

                def trunk_B(ck, t1c, t1p):
                    c0 = ck * 512
                    gcp = [sbg.tile([128, 2, 512], F8, tag=f"gcp{g}", name=f"gcp{g}")
                           for g in range(3)]
                    for mt in range(6):
                        fp1 = pcv.tile([128, 512], F32, tag="cv", name="cv")
                        nc.tensor.matmul(fp1[:], wf1p[:, :, 128 * mt:128 * (mt + 1)],
                                         t1p[:], start=True, stop=True,
                                         perf_mode=DR)
                        nc.scalar.activation(gcp[mt % 3][:, mt // 3, :], fp1[:],
                                             AF.Gelu)
                    t2c = [sbc.tile([96, 512], BF16, tag=f"t2c{i}", name=f"t2c{i}")
                           for i in range(2)]
                    for mi in range(2):
                        mo = 96 * mi
                        fp2 = pcv.tile([128, 512], F32, tag="cv", name="cv")
                        for g in range(3):
                            nc.tensor.matmul(fp2[:96, :], wf2p[g][:, :, mo:mo + 96],
                                             gcp[g][:], start=(g == 0), stop=(g == 2),
                                             perf_mode=DR)
                        nc.vector.scalar_tensor_tensor(
                            out=t2c[mi][:], in0=t1c[mi][:], scalar=gammav[:96, :],
                            in1=fp2[:96, :], op0=OP.mult, op1=OP.add)
                    for mi, (mo, ms) in enumerate(KB):
                        pp = pcv.tile([128, 512], F32, tag="cv", name="cv")
                        for i in range(2):
                            nc.tensor.matmul(pp[:ms, :], wpr_t[i][:, mo:mo + ms],
                                             t2c[i][:], start=(i == 0), stop=(i == 1))
                        oc = sbc.tile([128, 512], BF16, tag=f"oc{mi}", name=f"oc{mi}")
                        nc.vector.tensor_copy(oc[:ms, :], pp[:ms, :])
                        nc.sync.dma_start(d_out[mo:mo + ms, c0:c0 + 512],
                                          oc[:ms, :])

                prevA = None
                for ck in range(NCK + 1 if STAGE >= 4 else 0):
                    if ck < NCK:
                        curA = (ck, trunk_A(ck))
                    if prevA is not None:
                        pk, (t1c_, t1p_) = prevA
                        trunk_B(pk, t1c_, t1p_)
                    prevA = curA if ck < NCK else None

    nc.compile()
    return nc


_NC = None


def _get_nc():
    global _NC
    if _NC is None:
        _NC = build_nc()
    return _NC


def _prep_weights(q_w, q_dw_w, kv_w, kv_dw_w, linear_w, proj_w, ffn1_w, ffn2_w,
                  temperature, alpha, beta, gamma, delta):
    def pad_oc(w):  # [192 real oc, ic] -> [ic, 256 padded oc]
        out = np.zeros((C, CP), np.float32)
        for h in range(HEADS):
            out[:, CPH * h:CPH * h + CH] = w[CH * h:CH * (h + 1), :].T
        return out

    wq = pad_oc(np.asarray(q_w, np.float32))
    kv = np.asarray(kv_w, np.float32)
    wkv = np.concatenate([pad_oc(kv[:C]), pad_oc(kv[C:])], axis=1)

    def pad_dw(w):  # [192,1,3,3] -> [256, 9, 128] diag
        out = np.zeros((CP, 9, 128), np.float32)
        for h in range(HEADS):
            for j in range(CH):
                cp = CPH * h + j
                out[cp, :, cp % 128] = w[CH * h + j, 0].reshape(9)
        return out

    qdw = pad_dw(np.asarray(q_dw_w, np.float32))
    kvd = np.asarray(kv_dw_w, np.float32)
    kvdw = np.concatenate([pad_dw(kvd[:C]), pad_dw(kvd[C:])], axis=0)

    lin = np.asarray(linear_w, np.float32) * float(beta)
    wlin = np.zeros((CP, C), np.float32)
    for h in range(HEADS):
        wlin[CPH * h:CPH * h + CH, :] = lin[:, CH * h:CH * (h + 1)].T

    wf1 = np.asarray(ffn1_w, np.float32).T  # [192, 768]
    wf1p = wf1.reshape(2, 96, 768).transpose(1, 0, 2).copy()
    wf2 = (np.asarray(ffn2_w, np.float32) * float(delta)).T  # [768, 192]
    wf2p = wf2.reshape(2, 3, 128, C).transpose(1, 2, 0, 3).copy()
    wpr = np.asarray(proj_w, np.float32).T.copy()

    tempb = np.zeros((128, 2), np.float32)
    tv = np.asarray(temperature, np.float32).reshape(HEADS)
    for h in range(HEADS):
        tempb[64 * (h % 2):64 * (h % 2) + 64, h // 2] = tv[h]

    alphav = np.full((128, 1), float(alpha), np.float32)
    gammav = np.full((128, 1), float(gamma), np.float32)
    id128 = np.eye(128, dtype=np.float32)
    idrep = np.zeros((128, 64), np.float32)
    for p_ in range(128):
        idrep[p_, p_ % 64] = 1.0

    return {
        "wq": wq.astype(bf16), "wkv": wkv.astype(bf16),
        "qdw": qdw.astype(bf16), "kvdw": kvdw.astype(bf16),
        "wlin": wlin.astype(bf16), "wf1": wf1p.astype(f8np), "wf2": wf2p.astype(f8np), "wpr": wpr.astype(bf16),
        "tempb": tempb, "alpha": alphav, "gamma": gammav,
        "id128": id128, "idrep": idrep.astype(bf16),
    }


def kernel(**inputs):
    x = np.asarray(inputs["x"], np.float32)
    y = np.asarray(inputs["y"], np.float32)
    shared = _prep_weights(
        inputs["q_w"], inputs["q_dw_w"], inputs["kv_w"], inputs["kv_dw_w"],
        inputs["linear_w"], inputs["proj_w"], inputs["ffn1_w"], inputs["ffn2_w"],
        inputs["temperature"], inputs["alpha"], inputs["beta"],
        inputs["gamma"], inputs["delta"])

    in_maps = []
    for c in range(N_CORES):
        bi, s = c // 2, c % 2
        r0 = s * HLOC
        xe = np.zeros((C, ER, EC), np.float32)
        ye = np.zeros((C, ER, EC), np.float32)
        rlo, rhi = max(r0 - 1, 0), min(r0 + HLOC + 1, H)
        elo = rlo - (r0 - 1)
        xe[:, elo:elo + (rhi - rlo), 1:129] = x[bi, :, rlo:rhi, :]
        ye[:, elo:elo + (rhi - rlo), 1:129] = y[bi, :, rlo:rhi, :]
        m = dict(shared)
        m["xe"] = xe.reshape(C, NEXT).astype(bf16)
        m["ye"] = ye.reshape(C, NEXT).astype(bf16)
        m["yc"] = y[bi, :, r0:r0 + HLOC, :].reshape(C, NLOC).astype(bf16)
        in_maps.append(m)

    nc = _get_nc()
    res = run_bass_kernel_spmd(nc, in_maps, list(range(N_CORES)))
    out = np.empty((B, C, H, W), np.float32)
    for c in range(N_CORES):
        bi, s = c // 2, c % 2
        out[bi, :, s * HLOC:(s + 1) * HLOC, :] = \
            res.results[c]["out"].reshape(C, HLOC, W)
    return out

